# revision 1
# baseline (speedup 1.0000x reference)
"""Trainium2 Bass kernel for nn_AttentionNet (spatial-attention net).

Math restructure (host-side fold of the small projection weights):
    f = feat.reshape(B, C, N)                       N = 14*14 = 196
    query = w2v @ Wq + bq                           [S, M]
    scores[b,s,n] = sum_m query[s,m] * (f_b^T Wk + bk)[n,m]
                  = (query Wk^T) @ f_b  + const(s)  -> softmax over n is
                                                       invariant to const(s)
    Qk = query @ Wk^T                               [S, C]
    U  = V @ Wo^T ; P = U @ Wv^T                    [S, C]
    attended term  = sum_n softmax(Qk@f_b)[s,n] * (P@f_b)[s,n] + (U@bv + V@bo)[s]
    pool term      = (1/N) * sum_n (V @ f_b)[s,n]  computed as (V/N) @ rowsum_n(f_b)
    v2s[b,s] = pool + attended + kc[s]

Device work per core (16 of 128 batches, data parallel over 8 cores):
    2 big matmuls per batch in float32r (full PE rate at moving dim 392),
    softmax via ACT exp with fused row-sum, fused multiply+reduce on DVE,
    feat rowsums on DVE, final PE transpose + pool-matmul accumulation.
"""

import numpy as np

import concourse.bass as bass
import concourse.tile as tile
from concourse import mybir
from concourse.bass_utils import run_bass_kernel_spmd
from concourse.masks import make_identity

B, C, N = 128, 2048, 196
S = 312
NCORES = 8
BL = B // NCORES            # batches per core
NPAIR = BL // 2             # batch pairs per core (2 batches share a matmul)
CCH = C // 128              # contraction chunks
SCHUNKS = [(0, 128), (128, 128), (256, 56)]
SPAD = 3 * 128              # kc padded length
F32 = mybir.dt.float32
F32R = mybir.dt.float32r
AX = mybir.AxisListType
ALU = mybir.AluOpType
ACTF = mybir.ActivationFunctionType

_NC = None
_RESULTS = None  # last BassKernelResults, for profiling harnesses


def _build_kernel():
    nc = bass.Bass("TRN2", debug=False, target_bir_lowering=False,
                   num_devices=NCORES)
    feat = nc.dram_tensor("feat", [BL, C, N], F32R, kind="ExternalInput").ap()
    qpt = nc.dram_tensor("qpt", [128, CCH * 632], F32R, kind="ExternalInput").ap()
    vst = nc.dram_tensor("vst", [128, CCH * S], F32R, kind="ExternalInput").ap()
    kcp = nc.dram_tensor("kcp", [SPAD], F32R, kind="ExternalInput").ap()
    v2s = nc.dram_tensor("v2s", [BL, S], F32, kind="ExternalOutput").ap()

    with tile.TileContext(nc) as tc:
        from contextlib import ExitStack
        with ExitStack() as ctx:
            consts = ctx.enter_context(tc.tile_pool(name="consts", bufs=1))
            fpool = ctx.enter_context(tc.tile_pool(name="f", bufs=3))
            epool = ctx.enter_context(tc.tile_pool(name="e", bufs=3))
            scpool = ctx.enter_context(tc.tile_pool(name="sc", bufs=3))
            prpool = ctx.enter_context(tc.tile_pool(name="prod", bufs=3))
            spool = ctx.enter_context(tc.tile_pool(name="small", bufs=12))
            pss = ctx.enter_context(tc.tile_pool(name="pss", bufs=3, space="PSUM"))
            psw = ctx.enter_context(tc.tile_pool(name="psw", bufs=3, space="PSUM"))
            psout = ctx.enter_context(tc.tile_pool(name="psout", bufs=1, space="PSUM"))
            psjunk = ctx.enter_context(tc.tile_pool(name="psjunk", bufs=1, space="PSUM"))

            # One PSUM scratch cell reused (in PE program order) by every
            # wait-absorber matmul. A fresh pool tile per absorber would add a
            # PE slot-release wait; the f32r self-loading matmul only has ONE
            # hardware sync-wait slot (on its LDWEIGHTS), so each absorber must
            # carry exactly its one DMA/engine wait.
            junk = psjunk.tile([1, 8], F32)

            def absorb_waits(*aps):
                # (lhsT, rhs) pairs read [1,1] and [1,8] cells; f32r matmuls
                # need a moving dim of at least 8 to pass walrus's ISA check.
                for lhs1, rhs8 in aps:
                    nc.tensor.matmul(junk[:], lhs1, rhs8, start=True, stop=True)

            # Persistent SBUF state
            # Packed weight columns per c-chunk: [Qk s0 | Qk s1 | P s0 | P s1 |
            # Qk s2 | P s2] so every matmul group is one contiguous block.
            qp_sb = consts.tile([128, CCH, 632], F32R)
            vst_sb = consts.tile([128, CCH, S], F32R)      # (V/N)^T chunks
            ident = consts.tile([128, 128], F32)
            term2 = consts.tile([128, 3, BL], F32)        # [s_part, s_chunk, b]
            fp_all = consts.tile([128, CCH, BL], F32)     # rowsum_n(f) [c_part, c_chunk, b]

            # Weights are host-pre-shuffled to [128, k*cols] (contiguous
            # descriptor rows). The SP ring streams: first two weight chunks,
            # pair-0 feat, then the rest of the weights in three parts so
            # pair-0's c-major matmuls pace with chunk arrival.
            qpr = qpt.rearrange("p (k s) -> p k s", s=632)
            nc.sync.dma_start(out=qp_sb[:, 0:2], in_=qpr[:, 0:2])
            ones_sb = consts.tile([1, BL], F32R)
            kcrow = consts.tile([1, S], F32R)
            make_identity(nc, ident)
            absorb_waits((qp_sb[0:1, 0, 0:1], qp_sb[0:1, 0, 0:8]),
                         (ident[0:1, 0:1], ident[0:1, 0:8]))
            actdump = consts.tile([128, N], F32)

            def softmax_stage(scores_ps, w_ps, m, sc, pr):
                # scores_ps/w_ps: [m, 2, N] PSUM APs (may live in one tile at
                # different partition offsets for the packed tail chunk).
                sc_sb = scpool.tile([m, 2, N], F32, tag="sc")
                nc.scalar.copy(out=sc_sb[:], in_=scores_ps)
                negmax = spool.tile([m, 2], F32, tag="negmax")
                nc.vector.reduce_max(out=negmax, in_=sc_sb[:], axis=AX.X, negate=True)
                e = epool.tile([m, 2, N], F32, tag="e")
                den = spool.tile([m, 2], F32, tag="den")
                for h in range(2):
                    nc.scalar.activation(
                        out=e[:, h, :], in_=sc_sb[:, h, :], func=ACTF.Exp,
                        bias=negmax[:, h:h + 1], scale=1.0,
                        accum_out=den[:, h:h + 1],
                    )
                num = spool.tile([m, 2], F32, tag="num")
                prod = prpool.tile([m, 2, N], F32, tag="prod")
                nc.vector.tensor_mul(out=prod[:], in0=e[:], in1=w_ps)
                nc.vector.reduce_sum(out=num[:], in_=prod[:], axis=AX.X)
                rcp = spool.tile([m, 2], F32, tag="rcp")
                nc.vector.reciprocal(rcp, den[:])
                nc.vector.tensor_mul(
                    out=term2[0:m, sc, 2 * pr:2 * pr + 2],
                    in0=num[:], in1=rcp[:],
                )

            f1_prefetch = None
            for pr in range(NPAIR):
                if pr == 0:
                    # Interleave the remaining weight parts with pair-0/1 feat
                    # on the SP ring so c-major pairs 0-1 pace with delivery.
                    f_tile = fpool.tile([128, 2, CCH, N], F32R, name="f0", tag="f")
                    def _fdma(t, b, h):
                        nc.sync.dma_start(
                            out=t[:, h],
                            in_=feat[b].rearrange("(k p) n -> p k n", p=128))
                    _fdma(f_tile, 0, 0)
                    _fdma(f_tile, 1, 1)
                    nc.sync.dma_start(out=qp_sb[:, 2:6], in_=qpr[:, 2:6])
                    f1_prefetch = fpool.tile([128, 2, CCH, N], F32R, name="f1", tag="f")
                    _fdma(f1_prefetch, 2, 0)
                    nc.sync.dma_start(out=qp_sb[:, 6:11], in_=qpr[:, 6:11])
                    _fdma(f1_prefetch, 3, 1)
                    nc.sync.dma_start(out=qp_sb[:, 11:CCH], in_=qpr[:, 11:CCH])
                elif pr == 1:
                    f_tile = f1_prefetch
                else:
                    f_tile = fpool.tile([128, 2, CCH, N], F32R, name="fx", tag="f")
                    for h in range(2):
                        nc.sync.dma_start(
                            out=f_tile[:, h],
                            in_=feat[2 * pr + h].rearrange("(k p) n -> p k n", p=128),
                        )
                absorb_waits((f_tile[0:1, 0, 0, 0:1], f_tile[0:1, 0, 0, 0:8]),
                             (f_tile[0:1, 1, 0, 0:1], f_tile[0:1, 1, 0, 0:8]))
                # feat rowsums for the pooling term: chunks 0-10 on DVE (one
                # fused reduce), chunks 11-15 on ACT via Copy+accum_out.
                nc.vector.reduce_sum(
                    out=fp_all[:, 0:11, 2 * pr:2 * pr + 2].rearrange("p k h -> p h k"),
                    in_=f_tile[:, :, 0:11, :],
                    axis=AX.X,
                )
                for ck in range(11, CCH):
                    for h in range(2):
                        nc.scalar.activation(
                            out=actdump[:], in_=f_tile[:, h, ck, :], func=ACTF.Copy,
                            accum_out=fp_all[:, ck, 2 * pr + h:2 * pr + h + 1],
                        )
                # Column blocks of the packed weights: (psum rows, col0)
                groups = [(128, 0), (128, 256), (128, 128), (128, 384), (120, 512)]
                tiles = []
                for gi, (m, c0) in enumerate(groups):
                    pool = psw if gi in (1, 3) else pss
                    tiles.append(pool.tile([m, 2, N], F32, name=f"psg{gi}",
                                           tag="psw" if gi in (1, 3) else "pss"))
                if pr <= 1:
                    # c-major: consume weight chunks as the DMA parts land.
                    for ck in range(CCH):
                        for gi, (m, c0) in enumerate(groups):
                            nc.tensor.matmul(
                                tiles[gi][:], qp_sb[:, ck, c0:c0 + m],
                                f_tile[:, :, ck, :],
                                start=(ck == 0), stop=(ck == CCH - 1),
                            )
                else:
                    for gi, (m, c0) in enumerate(groups):
                        for ck in range(CCH):
                            nc.tensor.matmul(
                                tiles[gi][:], qp_sb[:, ck, c0:c0 + m],
                                f_tile[:, :, ck, :],
                                start=(ck == 0), stop=(ck == CCH - 1),
                            )
                softmax_stage(tiles[0][:], tiles[1][:], 128, 0, pr)
                softmax_stage(tiles[2][:], tiles[3][:], 128, 1, pr)
                softmax_stage(tiles[4][0:56], tiles[4][64:120], 56, 2, pr)

            nc.sync.dma_start(out=vst_sb, in_=vst.rearrange("p (k s) -> p k s", s=S))
            nc.sync.dma_start(out=kcrow, in_=kcp[0:S].rearrange("(p s) -> p s", p=1))
            nc.sync.dma_start(out=ones_sb, in_=kcp[S:S + BL].rearrange("(p s) -> p s", p=1))

            # Final: out[b, s] = term2^T + (V/N) @ rowsums, assembled in one
            # PSUM accumulation group.
            fp_r = consts.tile([128, CCH, BL], F32R)
            nc.vector.tensor_copy(fp_r[:], fp_all[:])
            absorb_waits((vst_sb[0:1, 0, 0:1], vst_sb[0:1, 0, 0:8]))
            out_ps = psout.tile([BL, S], F32)
            # Pool term + kc first (they only wait on the rowsums), then the
            # term2 transposes accumulate on top - keeps the last pair's
            # softmax chain off the PE critical path until the final ~0.3us.
            for ck in range(CCH):
                nc.tensor.matmul(
                    out_ps[:],
                    fp_r[:, ck],
                    vst_sb[:, ck],
                    start=(ck == 0), stop=False,
                )
            nc.tensor.matmul(
                out_ps[:],
                ones_sb[:],
                kcrow[:],
                start=False, stop=False,
            )
            for sc, (s0, m) in enumerate(SCHUNKS):
                nc.tensor.matmul(
                    out_ps[:, s0:s0 + m],
                    term2[0:m, sc, :],
                    ident[0:m, 0:m],
                    is_transpose=True,
                    start=False, stop=(sc == 2),
                )
            final_sb = consts.tile([BL, S], F32)
            nc.scalar.copy(out=final_sb[:], in_=out_ps[:])
            nc.sync.dma_start(out=v2s, in_=final_sb[:])

    _strip_pe_self_waits(nc)
    _hoist_excess_waits(nc)
    return nc


def _strip_pe_self_waits(nc):
    """Remove PE-on-PE semaphore waits from PE instructions.

    Tile's PSUM slot-reuse release emits a wait on the PE engine's own
    semaphore alongside the cross-engine reader wait. The self-wait can never
    guard a real hazard (PE reads only SBUF, writes only PSUM, and retires
    writes in order), but walrus allows exactly one sync wait on the f32r
    self-loading matmul, so the redundant wait breaks codegen.
    """
    def walk(b):
        for i in getattr(b, "instructions", []) or []:
            if str(getattr(i, "engine", "")).endswith("PE"):
                si = i.sync_info
                if si is not None and si.on_wait:
                    kept = [w for w in si.on_wait
                            if not str(w.ant_name).startswith("PE_")]
                    if len(kept) != len(si.on_wait):
                        si.on_wait = kept
        for sb in getattr(b, "blocks", []) or []:
            walk(sb)
    for b in nc.m.functions[0].blocks:
        walk(b)


def _hoist_excess_waits(nc):
    """Walrus allows a single sync wait per TPB instruction (one EVENTS slot).

    Tile sometimes emits 2+ waits on one instruction (e.g. a tile written by
    two DMAs, or a PSUM slot released by readers on two engines). Hoist all
    but one wait onto standalone EventSemaphore instructions inserted just
    before the consumer on the same engine - identical semantics, one wait
    per hardware instruction.
    """
    import bass_rust

    # Pick semaphore ids no instruction references (alloc_semaphore would
    # recycle ids of released-but-still-referenced Tile sems).
    used = set()
    for b in nc.m.functions[0].blocks:
        for i in b.instructions or []:
            si = i.sync_info
            if si is not None:
                for w in si.on_wait or []:
                    used.add(w.id)
                for u in si.on_update or []:
                    used.add(u.id)
    free = (i for i in range(255, -1, -1) if i not in used)
    sems = {}

    def sem_for(engine):
        key = str(engine)
        if key not in sems:
            sems[key] = (next(free), f"hoist_waits_{key.split('.')[-1]}")
        return sems[key]

    for b in nc.m.functions[0].blocks:
        insts = list(b.instructions or [])
        out = []
        changed = False
        for i in insts:
            si = i.sync_info
            waits = list(si.on_wait) if si is not None and si.on_wait else []
            if len(waits) > 1:
                for w in waits[:-1]:
                    ev = mybir.InstEventSemaphore(
                        name=f"hoist-{nc.next_id()}", ins=[], outs=[])
                    ev.engine = i.engine
                    # The update to a dedicated (never-waited) semaphore keeps
                    # CoreSim's event loop happy - every instruction must
                    # carry at least one sem update.
                    sem_id, sem_name = sem_for(i.engine)
                    upd = bass_rust.SyncUpdate(
                        sync_type="semaphore", id=sem_id, ant_name=sem_name,
                        update_mode="sem-inc", update_value=1)
                    ev.sync_info = bass_rust.SyncInfo(on_wait=[w], on_update=[upd])
                    out.append(ev)
                si.on_wait = [waits[-1]]
                changed = True
            out.append(i)
        if changed:
            b.instructions = out


def _get_nc():
    global _NC
    if _NC is None:
        _NC = _build_kernel()
    return _NC


def _precompute(w2v_att, Wq, bq, Wk, bk, Wv, bv, Wo, bo, V_att_final):
    d = lambda x: np.asarray(x, np.float64)
    query = d(w2v_att) @ d(Wq) + d(bq)              # [S, M]
    Qk = query @ d(Wk).T                            # [S, C]
    U = d(V_att_final) @ d(Wo).T                    # [S, M]
    P = U @ d(Wv).T                                 # [S, C]
    kc = U @ d(bv) + d(V_att_final) @ d(bo)         # [S]
    Vs = d(V_att_final) / N                         # [S, C]
    QkT, PT = Qk.T.astype(np.float32), P.T.astype(np.float32)
    # Tail block pads 8 zero columns so the P rows land on partition 64
    # (engine partition offsets must be 32-aligned).
    qpt = np.concatenate([QkT[:, 0:128], QkT[:, 128:256], PT[:, 0:128],
                          PT[:, 128:256], QkT[:, 256:312],
                          np.zeros((C, 8), np.float32), PT[:, 256:312]],
                         axis=1)                                  # [C, 632]
    # shuffle to [128, k*cols] so device loads are 128 contiguous descriptors
    qpt = np.ascontiguousarray(
        qpt.reshape(CCH, 128, 632).transpose(1, 0, 2).reshape(128, CCH * 632))
    vst = np.ascontiguousarray(Vs.T).astype(np.float32)      # [C, S]
    vst = np.ascontiguousarray(
        vst.reshape(CCH, 128, S).transpose(1, 0, 2).reshape(128, CCH * S))
    kcp = np.zeros(SPAD, np.float32)
    kcp[:S] = kc.astype(np.float32)
    kcp[S:S + BL] = 1.0
    return qpt, vst, kcp


def _ensure_ntff_hook():
    """If BASS_TRACE is set in the environment, run_bass_kernel_spmd imports
    antenv.axon_hooks, which this image lacks - graft the ctypes NTFF hook
    from trn_boot so tracing degrades gracefully instead of crashing."""
    import sys
    if "antenv.axon_hooks" in sys.modules:
        return
    try:
        import antenv.axon_hooks  # noqa: F401
    except ImportError:
        try:
            import types
            import trn_agent_boot.trn_boot as tb
            hook = tb._ntff_profile_via_ctypes("/opt/axon/libaxon_pjrt.so")
            m = types.ModuleType("antenv.axon_hooks")
            m.get_axon_ntff_profile_hook = lambda: hook
            sys.modules["antenv.axon_hooks"] = m
        except Exception:
            pass


def kernel(**inputs):
    global _RESULTS
    _ensure_ntff_hook()
    feat = np.ascontiguousarray(np.asarray(inputs["feat"], np.float32))
    f = feat.reshape(B, C, N)
    qpt, vst, kcp = _precompute(
        inputs["w2v_att"], inputs["Wq"], inputs["bq"], inputs["Wk"],
        inputs["bk"], inputs["Wv"], inputs["bv"], inputs["Wo"],
        inputs["bo"], inputs["V_att_final"],
    )
    nc = _get_nc()
    in_maps = [
        {
            "feat": np.ascontiguousarray(f[core * BL:(core + 1) * BL]),
            "qpt": qpt,
            "vst": vst,
            "kcp": kcp,
        }
        for core in range(NCORES)
    ]
    _RESULTS = run_bass_kernel_spmd(nc, in_maps, core_ids=list(range(NCORES)))
    return np.concatenate([r["v2s"] for r in _RESULTS.results], axis=0)

